# revision 33
# baseline (speedup 1.0000x reference)
"""Causal multi-head self-attention on 8 Trainium2 NeuronCores.

Problem: B=4, T=2048, C=1024, H=16 heads (d=64), fp32.
    q/k/v = x @ W{q,k,v}.T + b;  S = causal softmax(q k^T / sqrt(d));  y = (S v) @ Wo.T + bo

Sharding (8 cores): 2-D  (batch x head-group).
    core c -> batch b = c // 2, head-group g = c % 2 (8 heads / 512 features).
    Each core computes its batch's attention for its 8 heads plus the partial
    output projection against Wo[:, 512g:512g+512]; the host sums the two
    partials per batch and adds bo.

Device kernel (per core, identical SPMD program, Bass/Tile):
    The Tile scheduler executes each engine's instructions in emission order,
    so overlap is engineered explicitly: one software-pipelined loop over
    512-token query chunks in FORWARD order (attention for chunk ic only
    needs k/v chunks 0..ic).  The exp stream on the Scalar engine starts
    ~16us into the kernel (right after the chunk-0 x / Wk / Wq DMAs land),
    and the PE slack inside each ACT-bound attention window is filled by a
    FILLER queue: the next chunk's projection matmuls and the previous
    chunk's output-projection matmuls are emitted one instruction at a time
    between attention j-steps.  Fillers whose inputs arrive later (chunk-1
    x, chunk-0 v-projections) are queued behind enough ready work that the
    in-order PE stream never blocks on a DMA.

    att(ic,hp) per even/odd head pair: S^T tiles = k_j^T.T @ q^T into a
    2-bank PSUM pair (two K=64 matmuls on PE row-groups 0/64 run
    concurrently), fused exp over both heads (ACT, PSUM->SBUF), causal
    masking via one gpsimd affine_select on the boundary tile, PV matmuls
    O^T += [v|1].T @ E pipelined behind the exp stream.  Row 64 of the PV
    accumulator is the softmax denominator; normalization (lagged one
    head-pair) broadcasts its reciprocal via a gpsimd partition_broadcast
    fed by one batched DMA hop per head-pair.

    All matmul-feeding tiles are bf16 (x, Wq/Wk/Wv/Wo cast on host, output
    partials returned as bf16 and summed in fp32 on host): same 1 cycle/row
    PE issue as float32r, half the SBUF/DMA bytes, no 4x small-free-dim
    penalty (diagonal tiles trim to the live columns).  PSUM accumulation,
    biases and the normalization stay fp32; measured end-to-end rel-l2
    error ~6e-3 vs the 2e-2 gate.  Input DMAs are spread across the three
    DMA-capable queues (sync/scalar/gpsimd) ordered by first use, and a
    dummy exp pre-loads the ACT spline table during the initial DMA wait.

All host-side work is layout only (transpose/slice/replicate/bf16 cast) +
the final pairwise partial-sum; every FLOP of the reference runs on device
except the 8M-element partial-sum adds.
"""

import math
import os

import numpy as np

# persistent XLA/neuronx compile cache: makes repeat kernel() invocations
# from fresh processes skip the ~5 min helper-module compile when possible.
os.environ.setdefault("JAX_COMPILATION_CACHE_DIR", "/tmp/jax_comp_cache")

B, T, C, H = 4, 2048, 1024, 16
D = C // H  # 64
NCORES = 8
GROUPS = 2  # head-groups (tensor parallel dimension)
HG = H // GROUPS  # heads per core = 8
CG = C // GROUPS  # features per core = 512
SCALE = 1.0 / math.sqrt(D)
P = 128
TCH = 512  # query chunk / matmul free dim
NTCH = T // TCH  # 4
NHP = CG // P  # 4 head-pairs per core

_MODULE_CACHE = {}


def _build_module(mm_fast):
    from collections import deque

    import concourse.bass as bass  # noqa: F401
    import concourse.mybir as mybir
    import concourse.tile as tile
    from concourse import bacc

    f32 = mybir.dt.float32
    # mm_fast: bf16 matmul operands (host casts x/W to bf16); otherwise
    # float32r (same fp32 bits in DRAM, ~1.5e-4 matmul precision).
    bdt = mybir.dt.bfloat16 if mm_fast else mybir.dt.float32r
    odt = mybir.dt.bfloat16 if mm_fast else f32
    Exp = mybir.ActivationFunctionType.Exp

    nc = bacc.Bacc(None, target_bir_lowering=False)

    xt = nc.dram_tensor("xt", [C, T], bdt, kind="ExternalInput")
    wqt = nc.dram_tensor("wqt", [C, CG], bdt, kind="ExternalInput")
    wkt = nc.dram_tensor("wkt", [C, CG], bdt, kind="ExternalInput")
    wvt = nc.dram_tensor("wvt", [C, CG], bdt, kind="ExternalInput")
    wot = nc.dram_tensor("wot", [CG, C], bdt, kind="ExternalInput")
    bq2 = nc.dram_tensor("bq2", [P, NHP], f32, kind="ExternalInput")
    bk2 = nc.dram_tensor("bk2", [P, NHP], f32, kind="ExternalInput")
    bv1 = nc.dram_tensor("bv1", [1, CG], f32, kind="ExternalInput")
    out = nc.dram_tensor("out", [T, C], odt, kind="ExternalOutput")

    xt_r = xt.ap().rearrange("(cs p) t -> p cs t", p=P)  # [128, 8, 2048]
    wqt_r = wqt.ap().rearrange("(cs p) j -> p cs j", p=P)  # [128, 8, 512]
    wkt_r = wkt.ap().rearrange("(cs p) j -> p cs j", p=P)
    wvt_r = wvt.ap().rearrange("(cs p) j -> p cs j", p=P)
    wot_r = wot.ap().rearrange("(hp p) m -> p hp m", p=P)  # [128, 4, 1024]
    out_ap = out.ap()

    with tile.TileContext(nc) as tc:
        with (
            tc.tile_pool(name="wp", bufs=1) as wp,
            tc.tile_pool(name="persist", bufs=1) as persist,
            tc.tile_pool(name="smalls", bufs=1) as smalls,
            tc.tile_pool(name="qTp", bufs=2) as qTp,
            tc.tile_pool(name="xp", bufs=1) as xp,
            tc.tile_pool(name="ep", bufs=6) as ep,
            tc.tile_pool(name="otp", bufs=2) as otp,
            tc.tile_pool(name="obp", bufs=2) as obp,
            tc.tile_pool(name="npool", bufs=2) as npool,
            tc.tile_pool(name="ps3", bufs=2, space="PSUM") as ps3p,
            tc.tile_pool(name="psS", bufs=2, space="PSUM") as psS,
            tc.tile_pool(name="psO", bufs=2, space="PSUM") as psO,
        ):
            # ---- persistent k/v tiles (attention for chunk ic reads 0..ic)
            kT_t = [
                persist.tile([P, NHP, TCH], bdt, name=f"kT{c}")
                for c in range(NTCH)
            ]
            vx_t = [
                persist.tile([P, TCH // P, HG, D + 1], bdt, name=f"vx{c}")
                for c in range(NTCH)
            ]
            qT_t = {}
            xh = {}
            ot_t = {}
            tm_t = {}

            # ---- t0 DMAs.  Three DMA-capable queues (sync/scalar/gpsimd);
            # each carries whole tensors (sliced loads transfer slowly) in
            # first-use order: chunk-0 x + Wk + Wq land by ~16us, Wv/x1 by
            # ~22us, everything else well before its consumer.
            bqs = smalls.tile([P, NHP], f32)
            nc.sync.dma_start(bqs, bq2.ap())
            bks = smalls.tile([P, NHP], f32)
            nc.sync.dma_start(bks, bk2.ap())
            bv1s = smalls.tile([1, CG], f32)
            nc.sync.dma_start(bv1s, bv1.ap())

            def emit_x_load(ic, eng):
                xa = xp.tile([P, 4, TCH], bdt, tag=f"xa{ic}", name=f"xa{ic}")
                xb = xp.tile([P, 4, TCH], bdt, tag=f"xb{ic}", name=f"xb{ic}")
                tsl = slice(TCH * ic, TCH * (ic + 1))
                eng.dma_start(xa, xt_r[:, 0:4, tsl])
                eng.dma_start(xb, xt_r[:, 4:8, tsl])
                xh[ic] = (xa, xb)

            # Dependencies are tile-granular, so each weight is loaded as
            # two cs-half tiles: the first projection matmuls gate on 0.5MB,
            # not the full 1MB transfer (HBM is saturated during the lead).
            # The scalar-engine queue is measurably slow under contention
            # (~25-50GB/s), so it only carries the latest-needed x chunks.
            wk_h = [wp.tile([P, 4, CG], bdt, name=f"wk{h}") for h in range(2)]
            wq_h = [wp.tile([P, 4, CG], bdt, name=f"wq{h}") for h in range(2)]
            wv_h = [wp.tile([P, 4, CG], bdt, name=f"wv{h}") for h in range(2)]
            # gpsimd's queue is the fastest (~200GB/s measured): it carries
            # all six weight half-tiles in first-use order; sync carries x0
            # and x1; the slow scalar queue gets only the late x chunks.
            emit_x_load(0, nc.sync)
            nc.gpsimd.dma_start(wk_h[0], wkt_r[:, 0:4, :])
            nc.gpsimd.dma_start(wq_h[0], wqt_r[:, 0:4, :])
            nc.gpsimd.dma_start(wk_h[1], wkt_r[:, 4:8, :])
            nc.gpsimd.dma_start(wq_h[1], wqt_r[:, 4:8, :])
            nc.gpsimd.dma_start(wv_h[0], wvt_r[:, 0:4, :])
            nc.gpsimd.dma_start(wv_h[1], wvt_r[:, 4:8, :])
            emit_x_load(1, nc.sync)
            wots = wp.tile([P, NHP, C], bdt)
            nc.sync.dma_start(wots, wot_r)
            emit_x_load(2, nc.scalar)
            emit_x_load(3, nc.scalar)

            # bv broadcast [1,CG] -> [P,CG] on device (saves 254KB of DMA);
            # also used as the ones-init source.
            bvbs = smalls.tile([P, CG], f32)
            nc.gpsimd.partition_broadcast(bvbs, bv1s)

            # dummy exp: pre-loads the ACT spline table set (~2.7us) during
            # the initial DMA wait.
            escr = smalls.tile([P, NHP], f32)
            nc.scalar.activation(escr, bqs, Exp, scale=1.0)

            def xsrc(ic, cs):
                xa, xb = xh[ic]
                return (xa if cs < 4 else xb)[:, cs % 4, :]

            # ---------------- filler machinery ----------------
            # Each filler is a zero-arg closure emitting ~one PE instruction.
            # Attention j-steps pull fillers so projection/out-projection
            # matmuls fill PE slack under the ACT-bound exp stream.
            fillers = deque()
            gated = deque()  # chunk-1-x-dependent items, promoted at (0,hp2)
            p0q = deque()  # chunk-0 k/q tail items, drained during window 0

            def pull(n):
                for _ in range(n):
                    if p0q:
                        p0q.popleft()()
                    elif fillers:
                        fillers.popleft()()
                    else:
                        break

            def proj_items(ic):
                items = []
                state = {}

                def start_chunk():
                    qT_t[ic] = qTp.tile(
                        [P, NHP, TCH], bdt, tag="qT", name=f"qT{ic}"
                    )
                    # ones columns of v_ext (softmax-denominator trick)
                    # via a DVE tensor_scalar: 0*x + 1.
                    nc.vector.tensor_scalar(
                        vx_t[ic][:, :, :, D],
                        bvbs[:, 0 : (TCH // P) * HG].rearrange(
                            "p (a b) -> p a b", b=HG
                        ),
                        0.0,
                        1.0,
                        mybir.AluOpType.mult,
                        mybir.AluOpType.add,
                    )

                items.append(start_chunk)

                def make_kq(kind, jt, cs):
                    wts = (wk_h if kind == "k" else wq_h)[cs // 4]
                    bias = bks if kind == "k" else bqs
                    jsl = slice(P * jt, P * (jt + 1))

                    def f():
                        key = (kind, jt)
                        if cs == 0:
                            state[key] = ps3p.tile(
                                [P, TCH], f32, tag="pso3",
                                name=f"ps{kind}{ic}_{jt}",
                            )
                        nc.tensor.matmul(
                            state[key],
                            wts[:, cs % 4, jsl],
                            xsrc(ic, cs),
                            start=(cs == 0),
                            stop=(cs == 7),
                        )
                        if cs == 7:
                            dstT = kT_t[ic] if kind == "k" else qT_t[ic]
                            nc.vector.tensor_scalar_add(
                                dstT[:, jt, :], state.pop(key),
                                bias[:, jt : jt + 1],
                            )

                    return f

                def make_v(tt, cs):
                    def f():
                        key = ("v", tt)
                        if cs == 0:
                            state[key] = ps3p.tile(
                                [P, CG], f32, tag="pso3", name=f"psv{ic}_{tt}"
                            )
                        nc.tensor.matmul(
                            state[key],
                            xsrc(ic, cs)[:, P * tt : P * (tt + 1)],
                            wv_h[cs // 4][:, cs % 4, :],
                            start=(cs == 0),
                            stop=(cs == 7),
                        )
                        if cs == 7:
                            nc.vector.tensor_add(
                                vx_t[ic][:, tt, :, 0:D],
                                state.pop(key).rearrange(
                                    "p (h d) -> p h d", d=D
                                ),
                                bvbs.rearrange("p (h d) -> p h d", d=D),
                            )

                    return f

                for jt in range(NHP):
                    for cs in range(8):
                        items.append(make_kq("k", jt, cs))
                    for cs in range(8):
                        items.append(make_kq("q", jt, cs))
                    if jt == 0:
                        for tt in range(TCH // P):
                            for cs in range(8):
                                items.append(make_v(tt, cs))
                return items

            def outproj_items(ic):
                items = []
                state = {}

                def pre():
                    state["otn"] = ot_t.pop(ic)

                items.append(pre)

                def make_o(tt, mi, hp):
                    msl = slice(TCH * mi, TCH * (mi + 1))

                    def f():
                        if hp == 0 and mi == 0:
                            state["osb"] = obp.tile(
                                [P, C], odt, tag="osb", name=f"ob{ic}_{tt}"
                            )
                        if hp == 0:
                            state["ps"] = ps3p.tile(
                                [P, TCH], f32, tag="pso3",
                                name=f"ps3{ic}_{tt}_{mi}",
                            )
                        nc.tensor.matmul(
                            state["ps"],
                            state["otn"][:, hp, P * tt : P * (tt + 1)],
                            wots[:, hp, msl],
                            start=(hp == 0),
                            stop=(hp == NHP - 1),
                        )
                        if hp == NHP - 1:
                            nc.vector.tensor_copy(
                                state["osb"][:, msl], state.pop("ps")
                            )
                            if mi == C // TCH - 1:
                                trow = TCH * ic + P * tt
                                # alternate queues so the final writes drain
                                # in parallel instead of serializing behind
                                # one ring at kernel end.
                                eng = nc.sync if tt % 2 else nc.gpsimd
                                eng.dma_start(
                                    out_ap[trow : trow + P, :],
                                    state.pop("osb"),
                                )

                    return f

                for tt in range(TCH // P):
                    for mi in range(C // TCH):
                        for hp in range(NHP):
                            items.append(make_o(tt, mi, hp))
                return items

            # Last-chunk out-projection is split: head-pairs 0-2 accumulate
            # into SBUF partials as fillers under attention(3,3); only the
            # hp3 matmul + add + store remain after the final normalize.
            pp_t = {}

            def outproj3a_items(ic):
                items = []
                state = {}

                def make_a(tt, mi, hp):
                    msl = slice(TCH * mi, TCH * (mi + 1))

                    def f():
                        if hp == 0:
                            state["ps"] = ps3p.tile(
                                [P, TCH], f32, tag="pso3",
                                name=f"pa{tt}_{mi}",
                            )
                        nc.tensor.matmul(
                            state["ps"],
                            ot_t[ic][:, hp, P * tt : P * (tt + 1)],
                            wots[:, hp, msl],
                            start=(hp == 0),
                            stop=(hp == 2),
                        )
                        if hp == 2:
                            pp = obp.tile(
                                [P, TCH], f32, tag="pp", bufs=8,
                                name=f"pp{tt}_{mi}",
                            )
                            pp_t[(tt, mi)] = pp
                            nc.vector.tensor_copy(pp, state.pop("ps"))

                    return f

                for tt in range(TCH // P):
                    for mi in range(C // TCH):
                        for hp in range(3):
                            items.append(make_a(tt, mi, hp))
                return items

            def outproj3b(ic):
                otn = ot_t.pop(ic)
                for tt in range(TCH // P):
                    osb = obp.tile([P, C], odt, tag="osb", name=f"ob3_{tt}")
                    for mi in range(C // TCH):
                        msl = slice(TCH * mi, TCH * (mi + 1))
                        ps = ps3p.tile(
                            [P, TCH], f32, tag="pso3", name=f"pb{tt}_{mi}"
                        )
                        nc.tensor.matmul(
                            ps,
                            otn[:, 3, P * tt : P * (tt + 1)],
                            wots[:, 3, msl],
                            start=True,
                            stop=True,
                        )
                        nc.vector.tensor_add(
                            osb[:, msl], ps, pp_t.pop((tt, mi))
                        )
                    trow = TCH * ic + P * tt
                    eng = nc.sync if tt % 2 else nc.gpsimd
                    eng.dma_start(out_ap[trow : trow + P, :], osb)

            # ---------------- attention for one (chunk, head-pair) --------
            def attention(ic, hp, nf):
                if hp == 0:
                    ot_t[ic] = otp.tile(
                        [P, NHP, TCH], bdt, tag="ot", name=f"ot{ic}"
                    )
                    tm_t[ic] = npool.tile(
                        [D, NHP, TCH], bdt, tag="tmpn", name=f"tm{ic}"
                    )
                njt = 4 * (ic + 1)
                ps_oe = psO.tile([P, TCH], f32, tag="ps_o", name=f"poe{ic}_{hp}")
                ps_oo = psO.tile([P, TCH], f32, tag="ps_o", name=f"poo{ic}_{hp}")
                ps_os = (ps_oe, ps_oo)
                pend = []  # (jt, E) awaiting their PV matmuls

                def flush_pv2():
                    # two j-steps at once, grouped per head so consecutive
                    # matmuls hit the same PSUM bank.
                    grp, pend[:2] = pend[:2], []
                    for h01 in range(2):
                        for jt, ee in grp:
                            cj, lj = jt // 4, jt % 4
                            # columns < 128r of a diagonal tile are fully
                            # masked: zero contribution, so the PV matmul
                            # skips them (earlier j-tiles wrote them).
                            lo = max(0, P * (jt - 4 * ic))
                            nc.tensor.matmul(
                                ps_os[h01][0 : D + 1, lo:],
                                vx_t[cj][:, lj, 2 * hp + h01, :],
                                ee[:, h01, lo:],
                                start=(jt == 0),
                                stop=(jt == njt - 1),
                            )

                for jt in range(njt):
                    cj, lj = jt // 4, jt % 4
                    r = jt - 4 * ic  # >= 0 only for diagonal tiles
                    # columns < 128r are fully masked; shrink the S matmul
                    # to the live tail.  (float32r pays 4x below 256 free
                    # columns, bf16 runs 1 cycle/row at any width.)
                    if r <= 0:
                        lo2 = 0
                    elif mm_fast:
                        lo2 = P * r
                    else:
                        lo2 = min(P * r, TCH // 2)
                    psp = psS.tile(
                        [P, 2, TCH], f32, tag="psp", name=f"psp{ic}_{hp}_{jt}"
                    )
                    for h01 in range(2):
                        pb = 64 * h01
                        nc.tensor.matmul(
                            psp[:, h01, lo2:],
                            kT_t[cj][pb : pb + D, hp, P * lj : P * (lj + 1)],
                            qT_t[ic][pb : pb + D, hp, lo2:],
                            start=True,
                            stop=True,
                        )
                    ee = ep.tile(
                        [P, 2, TCH], bdt, tag="ee", name=f"ee{ic}_{hp}_{jt}"
                    )
                    if r <= 0:
                        nc.scalar.activation(ee, psp, Exp, scale=SCALE)
                    else:
                        # columns < 128r are fully masked: never computed
                        # (the PV matmul skips them too).
                        nc.scalar.activation(
                            ee[:, :, P * r :],
                            psp[:, :, P * r :],
                            Exp,
                            scale=SCALE,
                        )
                    if r >= 0:
                        # boundary 128 columns: keep where -p + f >= 0
                        # (f local to the slice starting at column 128r)
                        bsl = slice(P * r, P * (r + 1))
                        nc.gpsimd.affine_select(
                            out=ee[:, :, bsl],
                            in_=ee[:, :, bsl],
                            compare_op=mybir.AluOpType.is_ge,
                            fill=0.0,
                            base=0,
                            pattern=[[0, 2], [1, P]],
                            channel_multiplier=-1,
                        )
                    pend.append((jt, ee))
                    # pull before flushing so window-0's v-projection
                    # fillers are all emitted before the first PV reads vx.
                    pull(nf)
                    if len(pend) == 4:
                        flush_pv2()
                while pend:
                    flush_pv2()

                # Evacuate the PV accumulators to SBUF right away so the
                # PSUM banks recycle without waiting on the (high-latency)
                # normalization chain.
                ow2 = npool.tile(
                    [D + 1, 2, TCH], f32, tag="oraw", bufs=3, name=f"or{ic}_{hp}"
                )
                for h01 in range(2):
                    nc.vector.tensor_copy(
                        ow2[:, h01, :], ps_os[h01][0 : D + 1, :]
                    )
                return ow2

            def normalize(ic, hp, ow2):
                # rows 0..63 are O^T, row 64 the softmax sums.
                # partition_broadcast only reads physical partition 0
                # (base-64 APs return garbage on HW): DMA-hop the row.
                heng = nc.sync if ic >= 2 else nc.gpsimd
                stmp = npool.tile(
                    [1, 2, TCH], f32, tag="stmp", name=f"st{ic}_{hp}"
                )
                heng.dma_start(stmp, ow2[D : D + 1, :, :])
                # reciprocal on the 2x512 row first, then broadcast: the
                # chain is shorter and the DVE touches 64x fewer elements.
                if mm_fast:
                    nc.vector.reciprocal_approx_fast(stmp, stmp)
                else:
                    nc.vector.reciprocal(stmp, stmp)
                rb = npool.tile(
                    [D, 2, TCH], f32, tag="rb", name=f"rb{ic}_{hp}"
                )
                nc.gpsimd.partition_broadcast(rb, stmp)
                nc.vector.tensor_mul(
                    ot_t[ic][0:D, hp, :], ow2[0:D, 0, :], rb[:, 0, :]
                )
                # odd head lands on partitions 64..127 of ot; DVE lanes are
                # partition-fixed, so stage in tm and DMA-hop once per chunk.
                nc.vector.tensor_mul(
                    tm_t[ic][:, hp, :], ow2[0:D, 1, :], rb[:, 1, :]
                )
                if ic == NTCH - 1:
                    # per-hp hops so the split final out-projection can read
                    # head-pairs 0-2 while attention(3,3) still runs.
                    heng.dma_start(ot_t[ic][D:P, hp, :], tm_t[ic][:, hp, :])
                    if hp == NHP - 1:
                        tm_t.pop(ic)
                elif hp == NHP - 1:
                    heng.dma_start(ot_t[ic][D:P, :, :], tm_t.pop(ic))

            # ---------------- merged pipeline ----------------
            # Window 0 (attention on chunk 0) starts right after the chunk-0
            # k/q projections; the rest of proj(0) and all of proj(1) run as
            # fillers.  proj(1) depends on the chunk-1 x DMA (~22us) so it
            # stays gated until (0, hp2).
            p0 = proj_items(0)
            for it in p0[:17]:  # start_chunk + k/q jt0
                it()
            # v (32, Wv lands ~19us: drained by hp0's pulls before the
            # first PV flush) then k/q for jt 1..3 (16 items per jt).
            p0q.extend(p0[17:])
            gated.extend(proj_items(1))

            norm_q = []
            for ic in range(NTCH):
                if ic >= 1 and ic + 1 < NTCH:
                    fillers.extend(proj_items(ic + 1))
                njt = 4 * (ic + 1)
                for hp in range(NHP):
                    if ic == 0 and hp > 0:
                        # S for (0,hp) reads k/q jt<=hp: emit stragglers
                        # now so no reader precedes its writer.
                        while len(p0q) > 16 * (NHP - 1 - hp):
                            p0q.popleft()()
                        if hp == 2:
                            fillers.extend(gated)
                            gated.clear()
                    if ic == NTCH - 1 and hp == NHP - 1:
                        fillers.extend(outproj3a_items(ic))
                    avail = len(p0q) + len(fillers)
                    nf = -(-avail // (njt * (NHP - hp))) if avail else 0
                    if ic == 0 and hp == 0:
                        # hp0 must pull all 32 v items before its PV flush
                        nf = max(nf, 8)
                    norm_q.append((ic, hp, attention(ic, hp, nf)))
                    # last chunk normalizes with no lag so the final
                    # out-projection chain starts as early as possible.
                    keep = 0 if ic == NTCH - 1 else 1
                    while len(norm_q) > keep:
                        normalize(*norm_q.pop(0))
                    if hp == 0 and ic >= 1:
                        # safe only now: normalize(ic-1, 3) was just
                        # emitted, so outproj(ic-1) reads are ordered
                        # after every writer of ot[ic-1].
                        fillers.extend(outproj_items(ic - 1))
            while norm_q:
                normalize(*norm_q.pop(0))
            pull(len(p0q) + len(fillers))
            outproj3b(NTCH - 1)

    nc.compile()
    return nc


def get_module(mm_fast=True):
    key = bool(mm_fast)
    if key not in _MODULE_CACHE:
        _MODULE_CACHE[key] = _build_module(key)
    return _MODULE_CACHE[key]


def make_in_maps(x, Wq, bq, Wk, bk, Wv, bv, Wo, bo, mm_fast=True):
    import ml_dtypes

    mmdt = ml_dtypes.bfloat16 if mm_fast else np.float32
    x = np.asarray(x, dtype=np.float32)
    Wq = np.asarray(Wq, dtype=np.float32)
    Wk = np.asarray(Wk, dtype=np.float32)
    Wv = np.asarray(Wv, dtype=np.float32)
    Wo = np.asarray(Wo, dtype=np.float32)
    bq = np.asarray(bq, dtype=np.float32)
    bk = np.asarray(bk, dtype=np.float32)
    bv = np.asarray(bv, dtype=np.float32)

    in_maps = []
    for core in range(NCORES):
        b, g = core // GROUPS, core % GROUPS
        gs = slice(CG * g, CG * (g + 1))
        in_maps.append(
            {
                "xt": np.ascontiguousarray(x[b].T).astype(mmdt),
                "wqt": np.ascontiguousarray(Wq[gs, :].T).astype(mmdt),
                "wkt": np.ascontiguousarray(Wk[gs, :].T).astype(mmdt),
                "wvt": np.ascontiguousarray(Wv[gs, :].T).astype(mmdt),
                "wot": np.ascontiguousarray(Wo[:, gs].T).astype(mmdt),
                "bq2": np.ascontiguousarray(bq[gs].reshape(NHP, P).T),
                "bk2": np.ascontiguousarray(bk[gs].reshape(NHP, P).T),
                "bv1": np.ascontiguousarray(bv[gs].reshape(1, CG)),
            }
        )
    return in_maps


def combine_results(results, bo):
    bo = np.asarray(bo, dtype=np.float32)
    out = np.empty((B, T, C), dtype=np.float32)
    for b in range(B):
        out[b] = (
            np.asarray(results[GROUPS * b]["out"], dtype=np.float32)
            + np.asarray(results[GROUPS * b + 1]["out"], dtype=np.float32)
            + bo[None, :]
        )
    return out


def kernel(**inputs):
    from concourse.bass_utils import run_bass_kernel_spmd

    nc = get_module(mm_fast=True)
    in_maps = make_in_maps(
        inputs["x"],
        inputs["Wq"],
        inputs["bq"],
        inputs["Wk"],
        inputs["bk"],
        inputs["Wv"],
        inputs["bv"],
        inputs["Wo"],
        inputs["bo"],
        mm_fast=True,
    )
    res = run_bass_kernel_spmd(nc, in_maps, core_ids=list(range(NCORES)))
    return combine_results(res.results, inputs["bo"])
